# revision 1
# baseline (speedup 1.0000x reference)
"""CTC loss on 8 trn2 NeuronCores.

Design:
- Batch B=64 split 8/core for the memory-bound part: each core streams its
  own 27MB of predicts through ACT exp(+accum) for the log_softmax
  denominators, which factor out of the CTC DP entirely
  (loss = -(ln L + renorms - sum_t ln denom_t)).
- The T=128-step CTC DP runs in linear space with periodic renorm. The
  serial chain is split in half across core pairs: even cores run the
  FORWARD chain for the pair's 16 samples, odd cores the BACKWARD
  (suffix) chain, both as the *identical* SPMD program — the direction
  lives entirely in host-prepared data (s-axis reversed for backward,
  transition masks baked in as -1e30 logits, E_127 absorbed into the
  backward init). Both chains are 63 steps of 3 fused DVE ops + 1 final
  multiply; cores exchange chain states with a pairwise AllGather and
  combine L = sum_s alpha_63[s] * gamma_63[s].
"""

from contextlib import ExitStack

import numpy as np

import concourse.bacc as bacc
import concourse.tile as tile
import concourse.mybir as mybir
from concourse.ap import AP
from concourse.bass_utils import run_bass_kernel_spmd

B, T, C, L = 64, 128, 6625, 25
S = 2 * L + 1  # 51
M = 8          # cores
BS = B // M    # own samples per core (denominator stream)
PS = 2 * BS    # pair samples per core (DP chain)
NSTEP = 63
NSLOT = 64     # 63 steps + final-multiply slot
RENORM = 8
NREN = 8       # 7 in-chain renorms + 1 pre-final
ECH = 16       # chain slots per exp chunk
CHUNKS = [(0, 3313), (3313, 3312)]
F32 = mybir.dt.float32

_cached = {}


def _dup_free(ap, n):
    """AP reading the free range of `ap` n times: [.., (0,n), (step,cnt)]."""
    dims = [list(d) for d in ap.ap]
    return AP(ap.tensor, ap.offset, dims[:-1] + [[0, n]] + [dims[-1]])


def _rev_free(ap):
    """AP reading the innermost free dim of `ap` reversed."""
    dims = [list(d) for d in ap.ap]
    st, ct = dims[-1]
    return AP(ap.tensor, ap.offset + st * (ct - 1), dims[:-1] + [[-st, ct]])


def _strided2(ap, gap, n):
    """AP over `ap`'s tensor writing two n-wide blocks `gap` apart."""
    dims = [list(d) for d in ap.ap]
    return AP(ap.tensor, ap.offset, dims[:-1] + [[gap, 2], [1, n]])


def _build():
    if "nc" in _cached:
        return _cached["nc"]
    nc = bacc.Bacc(
        "TRN2", target_bir_lowering=False, debug=False, num_devices=M
    )
    x = nc.dram_tensor("x", [BS, T, C], F32, kind="ExternalInput").ap()
    gcat = nc.dram_tensor("gcat", [PS, NSLOT * 2 * S], F32,
                          kind="ExternalInput").ap()
    yinit = nc.dram_tensor("yinit", [PS, S], F32, kind="ExternalInput").ap()
    xpk = nc.dram_tensor("xpk", [PS, S + NREN], F32, kind="ExternalOutput").ap()
    dsum = nc.dram_tensor("dsum", [BS, 1], F32, kind="ExternalOutput").ap()

    EXP = mybir.ActivationFunctionType.Exp
    LN = mybir.ActivationFunctionType.Ln
    MULT = mybir.AluOpType.mult
    CW = 2 * S * ECH  # exp chunk width

    with tile.TileContext(nc) as tc, ExitStack() as ctx:
        cpool = ctx.enter_context(tc.tile_pool(name="consts", bufs=1))
        xpool = ctx.enter_context(tc.tile_pool(name="xs", bufs=5))
        epool = ctx.enter_context(tc.tile_pool(name="es", bufs=3))
        pspool = ctx.enter_context(tc.tile_pool(name="ps", bufs=1, space="PSUM"))
        dram = ctx.enter_context(tc.tile_pool(name="dram", bufs=1, space="DRAM"))

        # --- arm the first stream tiles, then the small inputs ---
        xt0 = xpool.tile([128, CHUNKS[0][1]], F32, tag="xt", name="xt0")
        nc.sync.dma_start(xt0[:], x[0, :, 0 : CHUNKS[0][1]])
        xt1 = xpool.tile([128, CHUNKS[1][1]], F32, tag="xt", name="xt1")
        nc.sync.dma_start(xt1[:], x[0, :, CHUNKS[1][0] : CHUNKS[1][0] + CHUNKS[1][1]])
        pre = [xt0, xt1]
        y_sb = cpool.tile([PS, S], F32)
        nc.sync.dma_start(y_sb[:], yinit)
        gts = [cpool.tile([PS, CW], F32, tag=f"gt{i}", name=f"gt{i}") for i in range(4)]
        for i in range(4):
            nc.sync.dma_start(gts[i][:], gcat[:, i * CW : (i + 1) * CW])

        # --- chunked exp of chain factors (DP starts after chunk 0) ---
        ets = [cpool.tile([PS, CW], F32, tag=f"et{i}", name=f"et{i}") for i in range(4)]
        for i in range(4):
            nc.scalar.activation(ets[i][:], gts[i][:], EXP)

        # --- DP chain: 63 steps of 3 fused DVE ops ---
        # wcat layout: [pad2 | w(51) | pad2 | wc(51)] = 106 cols
        wcat = cpool.tile([PS, 2 * S + 4], F32)
        u_t = cpool.tile([PS, S], F32)
        xpack = cpool.tile([PS, S + NREN], F32)  # [X(51) | ys(8)]
        inv = cpool.tile([PS, 1], F32)
        nc.vector.memset(wcat[:], 0.0)

        w_view = _strided2(wcat[:, 2 : 2 + S], 53, S)
        ys = xpack[:, S : S + NREN]
        jren = 0
        pending = False
        for k in range(1, NSTEP + 1):
            ci, off = (k - 1) // ECH, ((k - 1) % ECH) * 2 * S
            ek = ets[ci][:, off : off + 2 * S].rearrange(
                "p (two s) -> p two s", two=2
            )
            if pending:
                nc.vector.scalar_tensor_tensor(
                    w_view, _dup_free(y_sb[:], 2), inv[:], ek, MULT, MULT
                )
                pending = False
            else:
                nc.vector.tensor_mul(w_view, _dup_free(y_sb[:], 2), ek)
            nc.vector.tensor_add(u_t[:], wcat[:, 2 : 2 + S], wcat[:, 1 : 1 + S])
            nc.vector.tensor_add(y_sb[:], u_t[:], wcat[:, S + 2 : 2 * S + 2])
            if k % RENORM == 0:
                nc.vector.reduce_max(ys[:, jren : jren + 1], y_sb[:],
                                     axis=mybir.AxisListType.X)
                nc.vector.reciprocal(inv[:], ys[:, jren : jren + 1])
                pending = True
                jren += 1

        # final multiply (slot 64 A-half: fwd E_63 / bwd ones) + renorm
        nc.vector.reduce_max(ys[:, jren : jren + 1], y_sb[:],
                             axis=mybir.AxisListType.X)
        nc.vector.reciprocal(inv[:], ys[:, jren : jren + 1])
        jren += 1
        assert jren == NREN
        foff = (NSTEP % ECH) * 2 * S
        efin = ets[3][:, foff : foff + S]
        nc.vector.scalar_tensor_tensor(
            xpack[:, 0:S], y_sb[:], inv[:], efin, MULT, MULT
        )

        nc.sync.dma_start(xpk, xpack[:])

        lsum = pspool.tile([BS, 1], F32)

        # --- denominator stream (the memory-bound part) ---
        denp = cpool.tile([128, 2 * BS], F32)
        den_all = cpool.tile([128, BS], F32)
        ld_all = cpool.tile([128, BS], F32)
        for b in range(BS):
            for ci, (c0, cw) in enumerate(CHUNKS):
                if b == 0:
                    xt = pre[ci]
                else:
                    xt = xpool.tile([128, cw], F32, tag="xt")
                    nc.sync.dma_start(xt[:], x[b, :, c0 : c0 + cw])
                et2 = epool.tile([128, cw], F32, tag="et2")
                idx = 2 * b + ci
                nc.scalar.activation(
                    et2[:], xt[:], EXP, accum_out=denp[:, idx : idx + 1]
                )
            nc.vector.tensor_add(
                den_all[:, b : b + 1], denp[:, 2 * b : 2 * b + 1],
                denp[:, 2 * b + 1 : 2 * b + 2],
            )
            nc.scalar.activation(ld_all[:, b : b + 1], den_all[:, b : b + 1], LN)
        ones = cpool.tile([128, 1], F32)
        nc.vector.memset(ones[:], 1.0)
        nc.tensor.matmul(lsum[:], lhsT=ld_all[:], rhs=ones[:],
                         start=True, stop=True)
        loss_sb = cpool.tile([BS, 1], F32)
        nc.vector.tensor_copy(loss_sb[:], lsum[:])
        nc.sync.dma_start(dsum, loss_sb[:])

    nc.compile()
    _cached["nc"] = nc
    return nc


def _host_prep(predicts, labels, label_lengths):
    predicts = np.ascontiguousarray(np.asarray(predicts, dtype=np.float32))
    labels = np.asarray(labels).astype(np.int64)
    lens = np.asarray(label_lengths).astype(np.int64)

    ext = np.zeros((B, S), np.int64)
    ext[:, 1::2] = labels
    ext_sm2 = np.zeros((B, S), np.int64)
    ext_sm2[:, 2:] = ext[:, :-2]
    skip = ((ext != 0) & (ext != ext_sm2)).astype(np.float32)  # m[s]

    g = np.take_along_axis(predicts, ext[:, None, :], axis=2)  # [B,T,S] f32
    se = (2 * lens).astype(np.int64)
    for b in range(B):
        g[b, :, se[b] + 1 :] = -1e30  # s>2*len never feeds back

    endm = np.zeros((B, S), np.float32)
    endm[np.arange(B), se] = 1.0
    endm[np.arange(B), se - 1] = 1.0

    NEG = np.float32(-1e30)
    in_maps = []
    for m in range(M):
        p = m // 2
        sl = slice(16 * p, 16 * p + PS)       # pair samples
        gp, skp, enp = g[sl], skip[sl], endm[sl]
        gc = np.full((PS, NSLOT, 2, S), NEG, np.float32)
        yi = np.zeros((PS, S), np.float32)
        if m % 2 == 0:
            # forward: step k consumes E_{k-1}; A=g[k-1,s]; C=g[k-1,s'] if m[s'+2]
            for k in range(1, NSTEP + 1):
                gc[:, k - 1, 0, :] = gp[:, k - 1, :]
                cm = np.full((PS, S), NEG, np.float32)
                cm[:, : S - 2] = np.where(skp[:, 2:] > 0, gp[:, k - 1, : S - 2], NEG)
                gc[:, k - 1, 1, :] = cm
            gc[:, NSTEP, 0, :] = gp[:, NSTEP, :]  # final-mul slot: E_63
            yi[:, 0] = 1.0
            yi[:, 1] = 1.0
        else:
            # backward, s-reversed; init absorbs E_127; steps consume E_126..E_64
            gr = gp[:, :, ::-1]               # \hat g
            mr = skp[:, ::-1]                 # \hat m
            for k in range(1, NSTEP + 1):
                t = T - 2 - k                 # 125 .. 63; consumes E_{t+1}
                gc[:, k - 1, 0, :] = gr[:, t + 1, :]
                gc[:, k - 1, 1, :] = np.where(mr > 0, gr[:, t + 1, :], NEG)
            gc[:, NSTEP, 0, :] = 0.0          # final-mul slot: ones
            w = np.exp(gp[:, T - 1, :]) * enp
            wm = skp * w
            gm = w.copy()
            gm[:, : S - 1] += w[:, 1:]
            gm[:, : S - 2] += wm[:, 2:]
            yi[:] = gm[:, ::-1]
        in_maps.append({
            "x": np.ascontiguousarray(predicts[m * BS : (m + 1) * BS]),
            "gcat": np.ascontiguousarray(gc.reshape(PS, NSLOT * 2 * S)),
            "yinit": yi,
        })
    return in_maps


def _run(in_maps, trace=False):
    nc = _build()
    res = run_bass_kernel_spmd(nc, in_maps, list(range(M)), trace=trace)
    losses = np.zeros(B, np.float32)
    for p in range(M // 2):
        re_, ro_ = res.results[2 * p], res.results[2 * p + 1]
        xe, xo = re_["xpk"][:, 0:S], ro_["xpk"][:, 0:S]
        yse, yso = re_["xpk"][:, S:], ro_["xpk"][:, S:]
        lv = (xe * xo[:, ::-1]).sum(axis=1, dtype=np.float32)
        tot = (np.log(lv) + np.log(yse).sum(1, dtype=np.float32)
               + np.log(yso).sum(1, dtype=np.float32))
        losses[16 * p : 16 * p + BS] = re_["dsum"].reshape(BS) - tot[:BS]
        losses[16 * p + BS : 16 * p + 16] = ro_["dsum"].reshape(BS) - tot[BS:]
    losses = np.where(losses < 1e29, losses, 0.0).astype(np.float32)
    out = np.asarray(losses.mean(), dtype=np.float32)
    return out, res


def kernel(predicts, labels, label_lengths):
    in_maps = _host_prep(predicts, labels, label_lengths)
    out, _ = _run(in_maps, trace=False)
    return out


def kernel_traced(predicts, labels, label_lengths):
    in_maps = _host_prep(predicts, labels, label_lengths)
    return _run(in_maps, trace=True)



# revision 2
# speedup vs baseline: 1.0790x; 1.0790x over previous
"""CTC loss on 8 trn2 NeuronCores.

Design:
- Batch B=64 split 8/core for the memory-bound part: each core streams its
  own 27MB of predicts through ACT exp(+accum) for the log_softmax
  denominators, which factor out of the CTC DP entirely
  (loss = -(ln L + renorms - sum_t ln denom_t)).
- The denominator stream keeps ACT to a single table set (Exp only): the
  per-(t, chunk) partial sums sum_c exp(x[t,c]) are DMA'd out raw and the
  host does log().sum() over the 128x11 result — this removes the
  Exp<->Ln ACT_TABLE_LOAD ping-pong (16 x 1.28us) and the final
  ln/add/matmul tail. Samples 0-6 stream as one 3.4MB DMA + one EXP each;
  sample 7 is split into 4 small chunks so the post-last-byte tail is one
  ~1.7us EXP instead of ~3us.
- The T=128-step CTC DP runs in linear space with periodic renorm. The
  serial chain is split in half across core pairs: even cores run the
  FORWARD chain for the pair's 16 samples, odd cores the BACKWARD
  (suffix) chain, both as the *identical* SPMD program — the direction
  lives entirely in host-prepared data (s-axis reversed for backward,
  transition masks baked in as -1e30 logits, E_127 absorbed into the
  backward init). Both chains are 63 steps of 3 fused DVE ops + 1 final
  multiply; cores combine L = sum_s alpha_63[s] * gamma_63[s] on host.
"""

from contextlib import ExitStack

import numpy as np

import concourse.bacc as bacc
import concourse.tile as tile
import concourse.mybir as mybir
from concourse.ap import AP
from concourse.bass_utils import run_bass_kernel_spmd

B, T, C, L = 64, 128, 6625, 25
S = 2 * L + 1  # 51
M = 8          # cores
BS = B // M    # own samples per core (denominator stream)
PS = 2 * BS    # pair samples per core (DP chain)
NSTEP = 63
NSLOT = 64     # 63 steps + final-multiply slot
RENORM = 8
NREN = 8       # 7 in-chain renorms + 1 pre-final
GW = NSLOT * 2 * S  # gcat width (6528)
# sample 7 streamed in 4 small chunks to shrink the post-last-byte tail
TAILC = [(0, 1657), (1657, 1656), (3313, 1656), (4969, 1656)]
NDEN = (BS - 1) + len(TAILC)  # 11 accumulator columns
F32 = mybir.dt.float32

_cached = {}


def _dup_free(ap, n):
    """AP reading the free range of `ap` n times: [.., (0,n), (step,cnt)]."""
    dims = [list(d) for d in ap.ap]
    return AP(ap.tensor, ap.offset, dims[:-1] + [[0, n]] + [dims[-1]])


def _strided2(ap, gap, n):
    """AP over `ap`'s tensor writing two n-wide blocks `gap` apart."""
    dims = [list(d) for d in ap.ap]
    return AP(ap.tensor, ap.offset, dims[:-1] + [[gap, 2], [1, n]])


def _build():
    if "nc" in _cached:
        return _cached["nc"]
    nc = bacc.Bacc(
        "TRN2", target_bir_lowering=False, debug=False, num_devices=M
    )
    x = nc.dram_tensor("x", [BS, T, C], F32, kind="ExternalInput").ap()
    gcat = nc.dram_tensor("gcat", [PS, GW], F32, kind="ExternalInput").ap()
    yinit = nc.dram_tensor("yinit", [PS, S], F32, kind="ExternalInput").ap()
    xpk = nc.dram_tensor("xpk", [PS, S + NREN], F32, kind="ExternalOutput").ap()
    dsum = nc.dram_tensor("dsum", [T, NDEN], F32, kind="ExternalOutput").ap()

    EXP = mybir.ActivationFunctionType.Exp
    MULT = mybir.AluOpType.mult

    with tile.TileContext(nc) as tc, ExitStack() as ctx:
        cpool = ctx.enter_context(tc.tile_pool(name="consts", bufs=1))
        xpool = ctx.enter_context(tc.tile_pool(name="xs", bufs=3))
        dram = ctx.enter_context(tc.tile_pool(name="dram", bufs=1, space="DRAM"))

        # --- small inputs first (1.2us of the stream), then arm sample 0 ---
        y_sb = cpool.tile([PS, S], F32)
        nc.sync.dma_start(y_sb[:], yinit)
        gt = cpool.tile([PS, GW], F32)
        nc.sync.dma_start(gt[:], gcat)
        xt0 = xpool.tile([128, C], F32, tag="xt", name="xt0")
        nc.sync.dma_start(xt0[:], x[0])

        # chain factors: one exp, hidden under the sample-0 DMA
        et = cpool.tile([PS, GW], F32)
        nc.scalar.activation(et[:], gt[:], EXP)

        # --- DP chain: 63 steps of 3 fused DVE ops ---
        # wcat layout: [pad2 | w(51) | pad2 | wc(51)] = 106 cols
        wcat = cpool.tile([PS, 2 * S + 4], F32)
        u_t = cpool.tile([PS, S], F32)
        xpack = cpool.tile([PS, S + NREN], F32)  # [X(51) | ys(8)]
        inv = cpool.tile([PS, 1], F32)
        nc.vector.memset(wcat[:], 0.0)

        w_view = _strided2(wcat[:, 2 : 2 + S], 53, S)
        ys = xpack[:, S : S + NREN]
        jren = 0
        pending = False
        for k in range(1, NSTEP + 1):
            off = (k - 1) * 2 * S
            ek = et[:, off : off + 2 * S].rearrange(
                "p (two s) -> p two s", two=2
            )
            if pending:
                nc.vector.scalar_tensor_tensor(
                    w_view, _dup_free(y_sb[:], 2), inv[:], ek, MULT, MULT
                )
                pending = False
            else:
                nc.vector.tensor_mul(w_view, _dup_free(y_sb[:], 2), ek)
            nc.vector.tensor_add(u_t[:], wcat[:, 2 : 2 + S], wcat[:, 1 : 1 + S])
            nc.vector.tensor_add(y_sb[:], u_t[:], wcat[:, S + 2 : 2 * S + 2])
            if k % RENORM == 0:
                nc.vector.reduce_max(ys[:, jren : jren + 1], y_sb[:],
                                     axis=mybir.AxisListType.X)
                nc.vector.reciprocal(inv[:], ys[:, jren : jren + 1])
                pending = True
                jren += 1

        # final multiply (slot 64 A-half: fwd E_63 / bwd ones) + renorm
        nc.vector.reduce_max(ys[:, jren : jren + 1], y_sb[:],
                             axis=mybir.AxisListType.X)
        nc.vector.reciprocal(inv[:], ys[:, jren : jren + 1])
        jren += 1
        assert jren == NREN
        foff = NSTEP * 2 * S
        efin = et[:, foff : foff + S]
        nc.vector.scalar_tensor_tensor(
            xpack[:, 0:S], y_sb[:], inv[:], efin, MULT, MULT
        )

        nc.sync.dma_start(xpk, xpack[:])

        # --- denominator stream (the memory-bound part) ---
        # exp outputs are never read: all EXPs share one junk tile (in-order
        # on ACT), only the accumulator columns survive.
        junk = cpool.tile([128, C], F32)
        den = cpool.tile([128, NDEN], F32)
        for b in range(BS - 1):
            xt = xt0 if b == 0 else xpool.tile([128, C], F32, tag="xt")
            if b > 0:
                nc.sync.dma_start(xt[:], x[b])
            nc.scalar.activation(
                junk[:], xt[:], EXP, accum_out=den[:, b : b + 1]
            )
        for ci, (c0, cw) in enumerate(TAILC):
            xt = xpool.tile([128, cw], F32, tag="xt")
            nc.sync.dma_start(xt[:], x[BS - 1, :, c0 : c0 + cw])
            nc.scalar.activation(
                junk[:, 0:cw], xt[:], EXP,
                accum_out=den[:, BS - 1 + ci : BS + ci],
            )
        nc.sync.dma_start(dsum, den[:])

    nc.compile()
    _cached["nc"] = nc
    return nc


def _host_prep(predicts, labels, label_lengths):
    predicts = np.ascontiguousarray(np.asarray(predicts, dtype=np.float32))
    labels = np.asarray(labels).astype(np.int64)
    lens = np.asarray(label_lengths).astype(np.int64)

    ext = np.zeros((B, S), np.int64)
    ext[:, 1::2] = labels
    ext_sm2 = np.zeros((B, S), np.int64)
    ext_sm2[:, 2:] = ext[:, :-2]
    skip = ((ext != 0) & (ext != ext_sm2)).astype(np.float32)  # m[s]

    g = np.take_along_axis(predicts, ext[:, None, :], axis=2)  # [B,T,S] f32
    se = (2 * lens).astype(np.int64)
    for b in range(B):
        g[b, :, se[b] + 1 :] = -1e30  # s>2*len never feeds back

    endm = np.zeros((B, S), np.float32)
    endm[np.arange(B), se] = 1.0
    endm[np.arange(B), se - 1] = 1.0

    NEG = np.float32(-1e30)
    in_maps = []
    for m in range(M):
        p = m // 2
        sl = slice(16 * p, 16 * p + PS)       # pair samples
        gp, skp, enp = g[sl], skip[sl], endm[sl]
        gc = np.full((PS, NSLOT, 2, S), NEG, np.float32)
        yi = np.zeros((PS, S), np.float32)
        if m % 2 == 0:
            # forward: step k consumes E_{k-1}; A=g[k-1,s]; C=g[k-1,s'] if m[s'+2]
            for k in range(1, NSTEP + 1):
                gc[:, k - 1, 0, :] = gp[:, k - 1, :]
                cm = np.full((PS, S), NEG, np.float32)
                cm[:, : S - 2] = np.where(skp[:, 2:] > 0, gp[:, k - 1, : S - 2], NEG)
                gc[:, k - 1, 1, :] = cm
            gc[:, NSTEP, 0, :] = gp[:, NSTEP, :]  # final-mul slot: E_63
            yi[:, 0] = 1.0
            yi[:, 1] = 1.0
        else:
            # backward, s-reversed; init absorbs E_127; steps consume E_126..E_64
            gr = gp[:, :, ::-1]               # \hat g
            mr = skp[:, ::-1]                 # \hat m
            for k in range(1, NSTEP + 1):
                t = T - 2 - k                 # 125 .. 63; consumes E_{t+1}
                gc[:, k - 1, 0, :] = gr[:, t + 1, :]
                gc[:, k - 1, 1, :] = np.where(mr > 0, gr[:, t + 1, :], NEG)
            gc[:, NSTEP, 0, :] = 0.0          # final-mul slot: ones
            w = np.exp(gp[:, T - 1, :]) * enp
            wm = skp * w
            gm = w.copy()
            gm[:, : S - 1] += w[:, 1:]
            gm[:, : S - 2] += wm[:, 2:]
            yi[:] = gm[:, ::-1]
        in_maps.append({
            "x": np.ascontiguousarray(predicts[m * BS : (m + 1) * BS]),
            "gcat": np.ascontiguousarray(gc.reshape(PS, GW)),
            "yinit": yi,
        })
    return in_maps


def _run(in_maps, trace=False):
    nc = _build()
    res = run_bass_kernel_spmd(nc, in_maps, list(range(M)), trace=trace)
    losses = np.zeros(B, np.float32)
    for p in range(M // 2):
        re_, ro_ = res.results[2 * p], res.results[2 * p + 1]
        xe, xo = re_["xpk"][:, 0:S], ro_["xpk"][:, 0:S]
        yse, yso = re_["xpk"][:, S:], ro_["xpk"][:, S:]
        lv = (xe * xo[:, ::-1]).sum(axis=1, dtype=np.float32)
        tot = (np.log(lv) + np.log(yse).sum(1, dtype=np.float32)
               + np.log(yso).sum(1, dtype=np.float32))
        for half, r in ((0, re_), (1, ro_)):
            dnp = r["dsum"]  # [T, NDEN] raw chunk sums of exp
            dfull = np.empty((T, BS), np.float32)
            dfull[:, : BS - 1] = dnp[:, : BS - 1]
            dfull[:, BS - 1] = dnp[:, BS - 1 :].sum(axis=1, dtype=np.float32)
            dln = np.log(dfull).sum(axis=0, dtype=np.float32)  # [BS]
            losses[16 * p + 8 * half : 16 * p + 8 * half + BS] = (
                dln - tot[8 * half : 8 * half + BS]
            )
    losses = np.where(losses < 1e29, losses, 0.0).astype(np.float32)
    out = np.asarray(losses.mean(), dtype=np.float32)
    return out, res


def kernel(predicts, labels, label_lengths):
    in_maps = _host_prep(predicts, labels, label_lengths)
    out, _ = _run(in_maps, trace=False)
    return out


def kernel_traced(predicts, labels, label_lengths):
    in_maps = _host_prep(predicts, labels, label_lengths)
    return _run(in_maps, trace=True)


# revision 8
# speedup vs baseline: 1.1271x; 1.0446x over previous
"""CTC loss on 8 trn2 NeuronCores.

Design:
- Batch B=64 split 8/core for the memory-bound part: each core streams its
  own 27MB of predicts through ACT exp(+accum) for the log_softmax
  denominators, which factor out of the CTC DP entirely
  (loss = -(ln L + renorms - sum_t ln denom_t)).
- The denominator stream keeps ACT to a single table set (Exp only): the
  per-(t, chunk) partial sums sum_c exp(x[t,c]) are DMA'd out raw and the
  host does log().sum() over the 128x11 result — this removes the
  Exp<->Ln ACT_TABLE_LOAD ping-pong (16 x 1.28us) and the final
  ln/add/matmul tail. Samples 0-6 stream as one 3.4MB DMA + one EXP each;
  sample 7 is split into 4 small chunks so the post-last-byte tail is one
  ~1.7us EXP instead of ~3us.
- The T=128-step CTC DP runs in linear space with periodic renorm. The
  serial chain is split in half across core pairs: even cores run the
  FORWARD chain for the pair's 16 samples, odd cores the BACKWARD
  (suffix) chain, both as the *identical* SPMD program — the direction
  lives entirely in host-prepared data (s-axis reversed for backward,
  transition masks baked in as -1e30 logits, E_127 absorbed into the
  backward init). Both chains are 63 steps of 3 fused DVE ops + 1 final
  multiply; cores combine L = sum_s alpha_63[s] * gamma_63[s] on host.
"""

from contextlib import ExitStack

import numpy as np

import concourse.bacc as bacc
import concourse.tile as tile
import concourse.mybir as mybir
from concourse.ap import AP
from concourse.bass_utils import run_bass_kernel_spmd

B, T, C, L = 64, 128, 6625, 25
S = 2 * L + 1  # 51
M = 8          # cores
BS = B // M    # own samples per core (denominator stream)
PS = 2 * BS    # pair samples per core (DP chain)
NSTEP = 63
NSLOT = 64     # 63 steps + final-multiply slot
RENORM = 8
NREN = 8       # 7 in-chain renorms + 1 pre-final
GW = NSLOT * 2 * S  # gcat width (6528)
# tapered stream: full samples, then halves, then quarters — keeps ACT from
# queueing behind a long EXP once the DMA stream ends
PLAN = (
    [(b, 0, C) for b in range(5)]
    + [(b, c0, cw) for b in (5, 6) for c0, cw in ((0, 3313), (3313, 3312))]
    + [(7, 0, 1657), (7, 1657, 1656), (7, 3313, 1656), (7, 4969, 1656)]
)
NDEN = len(PLAN)      # 13 accumulator columns
NFULL = 5             # big-tile DMAs (dedicated pool)
DEN_SPLIT = 9         # cols [0,9) DMA'd out mid-stream, rest at the end
F32 = mybir.dt.float32

_cached = {}


def _dup_free(ap, n):
    """AP reading the free range of `ap` n times: [.., (0,n), (step,cnt)]."""
    dims = [list(d) for d in ap.ap]
    return AP(ap.tensor, ap.offset, dims[:-1] + [[0, n]] + [dims[-1]])


def _strided2(ap, gap, n):
    """AP over `ap`'s tensor writing two n-wide blocks `gap` apart."""
    dims = [list(d) for d in ap.ap]
    return AP(ap.tensor, ap.offset, dims[:-1] + [[gap, 2], [1, n]])


def _build():
    if "nc" in _cached:
        return _cached["nc"]
    nc = bacc.Bacc(
        "TRN2", target_bir_lowering=False, debug=False, num_devices=M
    )
    x = nc.dram_tensor("x", [BS, T, C], F32, kind="ExternalInput").ap()
    gcat = nc.dram_tensor("gcat", [PS, GW], F32, kind="ExternalInput").ap()
    yinit = nc.dram_tensor("yinit", [PS, S], F32, kind="ExternalInput").ap()
    xpk = nc.dram_tensor("xpk", [PS, S + NREN], F32, kind="ExternalOutput").ap()
    dsum = nc.dram_tensor("dsum", [T, NDEN], F32, kind="ExternalOutput").ap()

    EXP = mybir.ActivationFunctionType.Exp
    MULT = mybir.AluOpType.mult

    with tile.TileContext(nc) as tc, ExitStack() as ctx:
        cpool = ctx.enter_context(tc.tile_pool(name="consts", bufs=1))
        xpool = ctx.enter_context(tc.tile_pool(name="xs", bufs=3))
        hpool = ctx.enter_context(tc.tile_pool(name="halves", bufs=2))
        qpool = ctx.enter_context(tc.tile_pool(name="quarters", bufs=4))
        spool = ctx.enter_context(tc.tile_pool(name="scratch", bufs=1))

        # --- arm sample 0, then the small chain inputs ---
        xt0 = xpool.tile([128, C], F32, tag="xt", name="xt0")
        nc.sync.dma_start(xt0[:], x[0])
        y_sb = cpool.tile([PS, S], F32)
        nc.sync.dma_start(y_sb[:], yinit)
        gt = spool.tile([PS, GW], F32, tag="sc", name="gt")
        nc.sync.dma_start(gt[:], gcat)

        # chain factors: one exp, hidden under the sample-0 DMA
        et = cpool.tile([PS, GW], F32)
        nc.scalar.activation(et[:], gt[:], EXP)

        # exp outputs of the denominator stream are never read; the shared
        # junk tile reuses gt's buffer (gt is dead once `et` is computed)
        junk = spool.tile([128, C], F32, tag="sc", name="junk")

        # --- DP chain: 63 steps of 3 fused DVE ops ---
        # wcat layout: [pad2 | w(51) | pad2 | wc(51)] = 106 cols
        wcat = cpool.tile([PS, 2 * S + 4], F32)
        u_t = cpool.tile([PS, S], F32)
        xpack = cpool.tile([PS, S + NREN], F32)  # [X(51) | ys(8)]
        inv = cpool.tile([PS, 1], F32)
        nc.vector.memset(wcat[:], 0.0)

        w_view = _strided2(wcat[:, 2 : 2 + S], 53, S)
        ys = xpack[:, S : S + NREN]
        jren = 0
        pending = False
        for k in range(1, NSTEP + 1):
            off = (k - 1) * 2 * S
            ek = et[:, off : off + 2 * S].rearrange(
                "p (two s) -> p two s", two=2
            )
            if pending:
                nc.vector.scalar_tensor_tensor(
                    w_view, _dup_free(y_sb[:], 2), inv[:], ek, MULT, MULT
                )
                pending = False
            else:
                nc.vector.tensor_mul(w_view, _dup_free(y_sb[:], 2), ek)
            nc.vector.tensor_add(u_t[:], wcat[:, 2 : 2 + S], wcat[:, 1 : 1 + S])
            nc.vector.tensor_add(y_sb[:], u_t[:], wcat[:, S + 2 : 2 * S + 2])
            if k % RENORM == 0:
                nc.vector.reduce_max(ys[:, jren : jren + 1], y_sb[:],
                                     axis=mybir.AxisListType.X)
                nc.vector.reciprocal(inv[:], ys[:, jren : jren + 1])
                pending = True
                jren += 1

        # final multiply (slot 64 A-half: fwd E_63 / bwd ones) + renorm
        nc.vector.reduce_max(ys[:, jren : jren + 1], y_sb[:],
                             axis=mybir.AxisListType.X)
        nc.vector.reciprocal(inv[:], ys[:, jren : jren + 1])
        jren += 1
        assert jren == NREN
        foff = NSTEP * 2 * S
        efin = et[:, foff : foff + S]
        nc.vector.scalar_tensor_tensor(
            xpack[:, 0:S], y_sb[:], inv[:], efin, MULT, MULT
        )

        # --- denominator stream (the memory-bound part) ---
        # Sync-queue dispatch order matters: all stream DMAs go first; the
        # dsum/xpk out-DMAs (whose waits would block the FIFO) come last.
        den = cpool.tile([128, NDEN], F32)
        for i, (b, c0, cw) in enumerate(PLAN):
            if i == 0:
                xt = xt0
            else:
                if cw == C:
                    xt = xpool.tile([128, cw], F32, tag="xt")
                elif cw > 2000:
                    xt = hpool.tile([128, cw], F32, tag="ht")
                else:
                    xt = qpool.tile([128, cw], F32, tag="qt")
                nc.sync.dma_start(xt[:], x[b, :, c0 : c0 + cw])
            nc.scalar.activation(
                junk[:, 0:cw], xt[:], EXP, accum_out=den[:, i : i + 1]
            )
        nc.sync.dma_start(dsum[:, 0:DEN_SPLIT], den[:, 0:DEN_SPLIT])
        nc.sync.dma_start(xpk, xpack[:])
        nc.sync.dma_start(dsum[:, DEN_SPLIT:], den[:, DEN_SPLIT:])

    nc.compile()
    _cached["nc"] = nc
    return nc


def _host_prep(predicts, labels, label_lengths):
    predicts = np.ascontiguousarray(np.asarray(predicts, dtype=np.float32))
    labels = np.asarray(labels).astype(np.int64)
    lens = np.asarray(label_lengths).astype(np.int64)

    ext = np.zeros((B, S), np.int64)
    ext[:, 1::2] = labels
    ext_sm2 = np.zeros((B, S), np.int64)
    ext_sm2[:, 2:] = ext[:, :-2]
    skip = ((ext != 0) & (ext != ext_sm2)).astype(np.float32)  # m[s]

    g = np.take_along_axis(predicts, ext[:, None, :], axis=2)  # [B,T,S] f32
    se = (2 * lens).astype(np.int64)
    for b in range(B):
        g[b, :, se[b] + 1 :] = -1e30  # s>2*len never feeds back

    endm = np.zeros((B, S), np.float32)
    endm[np.arange(B), se] = 1.0
    endm[np.arange(B), se - 1] = 1.0

    NEG = np.float32(-1e30)
    in_maps = []
    for m in range(M):
        p = m // 2
        sl = slice(16 * p, 16 * p + PS)       # pair samples
        gp, skp, enp = g[sl], skip[sl], endm[sl]
        gc = np.full((PS, NSLOT, 2, S), NEG, np.float32)
        yi = np.zeros((PS, S), np.float32)
        if m % 2 == 0:
            # forward: step k consumes E_{k-1}; A=g[k-1,s]; C=g[k-1,s'] if m[s'+2]
            for k in range(1, NSTEP + 1):
                gc[:, k - 1, 0, :] = gp[:, k - 1, :]
                cm = np.full((PS, S), NEG, np.float32)
                cm[:, : S - 2] = np.where(skp[:, 2:] > 0, gp[:, k - 1, : S - 2], NEG)
                gc[:, k - 1, 1, :] = cm
            gc[:, NSTEP, 0, :] = gp[:, NSTEP, :]  # final-mul slot: E_63
            yi[:, 0] = 1.0
            yi[:, 1] = 1.0
        else:
            # backward, s-reversed; init absorbs E_127; steps consume E_126..E_64
            gr = gp[:, :, ::-1]               # \hat g
            mr = skp[:, ::-1]                 # \hat m
            for k in range(1, NSTEP + 1):
                t = T - 2 - k                 # 125 .. 63; consumes E_{t+1}
                gc[:, k - 1, 0, :] = gr[:, t + 1, :]
                gc[:, k - 1, 1, :] = np.where(mr > 0, gr[:, t + 1, :], NEG)
            gc[:, NSTEP, 0, :] = 0.0          # final-mul slot: ones
            w = np.exp(gp[:, T - 1, :]) * enp
            wm = skp * w
            gm = w.copy()
            gm[:, : S - 1] += w[:, 1:]
            gm[:, : S - 2] += wm[:, 2:]
            yi[:] = gm[:, ::-1]
        in_maps.append({
            "x": np.ascontiguousarray(predicts[m * BS : (m + 1) * BS]),
            "gcat": np.ascontiguousarray(gc.reshape(PS, GW)),
            "yinit": yi,
        })
    return in_maps


def _run(in_maps, trace=False):
    nc = _build()
    res = run_bass_kernel_spmd(nc, in_maps, list(range(M)), trace=trace)
    losses = np.zeros(B, np.float32)
    for p in range(M // 2):
        re_, ro_ = res.results[2 * p], res.results[2 * p + 1]
        xe, xo = re_["xpk"][:, 0:S], ro_["xpk"][:, 0:S]
        yse, yso = re_["xpk"][:, S:], ro_["xpk"][:, S:]
        lv = (xe * xo[:, ::-1]).sum(axis=1, dtype=np.float32)
        tot = (np.log(lv) + np.log(yse).sum(1, dtype=np.float32)
               + np.log(yso).sum(1, dtype=np.float32))
        for half, r in ((0, re_), (1, ro_)):
            dnp = r["dsum"]  # [T, NDEN] raw chunk sums of exp
            dfull = np.zeros((T, BS), np.float32)
            for i, (b, _, _) in enumerate(PLAN):
                dfull[:, b] += dnp[:, i]
            dln = np.log(dfull).sum(axis=0, dtype=np.float32)  # [BS]
            losses[16 * p + 8 * half : 16 * p + 8 * half + BS] = (
                dln - tot[8 * half : 8 * half + BS]
            )
    losses = np.where(losses < 1e29, losses, 0.0).astype(np.float32)
    out = np.asarray(losses.mean(), dtype=np.float32)
    return out, res


def kernel(predicts, labels, label_lengths):
    in_maps = _host_prep(predicts, labels, label_lengths)
    out, _ = _run(in_maps, trace=False)
    return out


def kernel_traced(predicts, labels, label_lengths):
    in_maps = _host_prep(predicts, labels, label_lengths)
    return _run(in_maps, trace=True)
